# revision 2
# baseline (speedup 1.0000x reference)
"""Trainium2 Bass kernel v2: 1x1-conv GEMM + GroupNorm + HardTanh.

Reference computation (per sample b):
    y = weight @ x[b]                        # [512, 256] @ [256, 3136]
    groupnorm over 32 groups of 16 channels  # stats over (16, 3136)
    y = y * gamma + beta ; clip(y, -2, 2)

Data-parallel over batch: 4 samples/core x 8 cores, no cross-core comms.

Per-core program (16 chunks = 4 samples x 4 output-channel chunks):
 - fp16 GEMM (full PE rate). Host appends a sum(x, axis=f) column to x, so
   column 3136 of the last matmul tile yields 3136*mean per channel exactly.
 - Pass 1 frees PSUM fast: merged multi-bank copies PSUM->SBUF fp16 split
   across ACT/DVE/Pool so the next chunk's matmuls never wait on the
   normalization chain.
 - Pass 2: one DVE scalar_tensor_tensor square with accum_out gives
   sum(y^2) for the whole row in one instruction (2-byte SBUF operands).
 - Group reduce of [sum(y^2), 3136*mean] is one tiny PE matmul against a
   block-diagonal 1/(16*3136) matrix into PSUM bank 7.
 - Short chain: var = E[y^2]-mean^2, rstd, s = rstd*gamma*63.5; affine
   emits int8 directly (saturation = hardtanh clamp since 2*63.5 = 127),
   split across ACT (y*s+bv) and DVE/Pool ((y-mean)*s).
 - int8 store; host dequantizes /63.5 and clips.

Emission is software-pipelined: chunk k emits its matmuls+pass1+pass2,
the PREVIOUS chunk's group-matmul goes between the two matmul streams,
and the previous chunk's chain+affine+store follow, so no engine queue
head-of-line-blocks on a not-yet-ready dependency.
"""

import os
import sys

sys.path.insert(0, "/opt/trn_rl_repo")

STAGE = int(os.environ.get("KV2_STAGE", "4"))  # 1=mm 2=+p1/p2 3=+agg/chain 4=full

import numpy as np

import concourse.bacc as bacc
import concourse.mybir as mybir
import concourse.tile as tile
from concourse.bass_utils import run_bass_kernel_spmd

# Problem shape (hardcoded per contest contract)
B, CIN, COUT, H, W = 32, 256, 512, 56, 56
HW = H * W  # 3136
G = 32
GSIZE = COUT // G  # 16
EPS = 1e-5
QS = 63.5  # int8 quantization scale: clip(y,-2,2)*63.5 in [-127, 127]

N_CORES = 8
BPC = B // N_CORES  # 4 samples per core
KC = CIN // 128  # 2 contraction chunks
OC = COUT // 128  # 4 output-channel chunks
NT = 7  # psum banks (tiles) per row
NTS = HW // NT  # 448 columns per tile
NCHUNK = BPC * OC  # 16

# pass-1 copy groups: (engine, [banks]) ; pass-3 affine groups likewise.
# Contiguous banks -> one merged strided instruction.
P1_GROUPS = [("scalar", (0, 1, 2)), ("vector", (3, 4)), ("gpsimd", (5, 6))]
P3_GROUPS = [("scalar", (0, 1, 2)), ("vector", (3,)), ("gpsimd", (4, 5, 6))]

_NC_CACHE = None


def _build_program():
    f32 = mybir.dt.float32
    f16 = mybir.dt.float16
    i8 = mybir.dt.int8
    Act = mybir.ActivationFunctionType
    Alu = mybir.AluOpType

    nc = bacc.Bacc("TRN2", target_bir_lowering=False, debug=False)

    HW1 = HW + 1  # + host-computed sum(x) column
    x_d = nc.dram_tensor("x", [BPC, CIN, HW1], f16, kind="ExternalInput")
    wt_d = nc.dram_tensor("wt", [CIN, COUT], f16, kind="ExternalInput")
    gamma_d = nc.dram_tensor("gamma", [COUT], f32, kind="ExternalInput")
    beta_d = nc.dram_tensor("beta", [COUT], f32, kind="ExternalInput")
    agg_d = nc.dram_tensor("agg", [128, 128], f32, kind="ExternalInput")
    out_d = nc.dram_tensor("out", [BPC, COUT, HW], i8, kind="ExternalOutput")

    with tile.TileContext(nc) as tc:
        with (
            tc.tile_pool(name="singles", bufs=1) as singles,
            tc.tile_pool(name="xp", bufs=3) as xp,
            tc.tile_pool(name="y16p", bufs=3) as y16p,
            tc.tile_pool(name="y8p", bufs=2) as y8p,
            tc.tile_pool(name="small", bufs=4) as small,
            tc.tile_pool(name="psp", bufs=1, space="PSUM") as psp,
        ):
            # ---- one-time loads ----------------------------------------
            XQ = 4  # x loaded in 4 column-range pieces so matmuls start early
            QW = 784  # first 3 quarters; last is 785 wide (includes sum col)

            def load_x_quarter(x_tile, b, q):
                lo = q * QW
                hi = HW1 if q == XQ - 1 else (q + 1) * QW
                nc.sync.dma_start(
                    out=x_tile[:, :, lo:hi],
                    in_=x_d.ap()[b, :, lo:hi].rearrange("(c p) f -> p c f", p=128),
                )

            x_tiles = [xp.tile([128, KC, HW1], f16, tag="x", name="x0")]
            load_x_quarter(x_tiles[0], 0, 0)
            wt_sb = singles.tile([128, KC, COUT], f16)
            nc.sync.dma_start(
                out=wt_sb, in_=wt_d.ap().rearrange("(c p) m -> p c m", p=128)
            )
            gamma_sb = singles.tile([128, OC], f32)  # gamma * 63.5 (host)
            nc.gpsimd.dma_start(
                out=gamma_sb, in_=gamma_d.ap().rearrange("(c p) -> p c", p=128)
            )
            beta_sb = singles.tile([128, OC], f32)  # beta * 63.5 (host)
            nc.gpsimd.dma_start(
                out=beta_sb, in_=beta_d.ap().rearrange("(c p) -> p c", p=128)
            )
            agg_sb = singles.tile([128, 128], f32)  # blockdiag 1/(16*3136)
            nc.gpsimd.dma_start(out=agg_sb, in_=agg_d.ap())
            for q in range(1, XQ):
                load_x_quarter(x_tiles[0], 0, q)

            eps_sb = singles.tile([128, 1], f32)
            nc.vector.memset(eps_sb, EPS)

            # y^2 scratch reused by every chunk's accumulate instruction
            y2s = singles.tile([128, HW], f16)

            # all 8 psum banks as one tile: banks 0-6 = row tiles, bank 7 =
            # group-aggregate results (alternating column range per chunk)
            ps = psp.tile([128, 8 * 512], f32)

            def bank(j):
                return ps[:, j * 512 : j * 512 + NTS]

            def bank_view(banks, width=NTS):
                lo, hi = banks[0], banks[-1]
                v = ps[:, lo * 512 : (hi + 1) * 512]
                v = v.rearrange("p (t c) -> p t c", c=512)
                return v[:, :, 0:width]

            # per-chunk state carried between pipeline stages
            state = {}

            def emit_matmuls(k):
                b, oc = divmod(k, OC)
                x_sb = x_tiles[b]
                osl = slice(oc * 128, (oc + 1) * 128)
                for c in range(KC):
                    for j in range(NT):
                        ncols = NTS + 1 if j == NT - 1 else NTS
                        fsl = slice(j * NTS, j * NTS + ncols)
                        nc.tensor.matmul(
                            ps[:, j * 512 : j * 512 + ncols],
                            wt_sb[:, c, osl],
                            x_sb[:, c, fsl],
                            start=(c == 0),
                            stop=(c == KC - 1),
                        )

            def emit_pass1(k):
                y16 = y16p.tile([128, NT, NTS], f16, tag="y16")
                st = small.tile([128, 2], f32, tag="st")
                for eng, banks in P1_GROUPS:
                    src = bank_view(banks)
                    dst = y16[:, banks[0] : banks[-1] + 1, :]
                    if eng == "scalar":
                        nc.scalar.activation(out=dst, in_=src, func=Act.Copy)
                    elif eng == "vector":
                        nc.vector.tensor_copy(out=dst, in_=src)
                    else:
                        nc.gpsimd.tensor_copy(out=dst, in_=src)
                # mean column: psum bank6 col 448 -> stk col 1
                nc.scalar.activation(
                    out=st[:, 1:2],
                    in_=ps[:, 6 * 512 + NTS : 6 * 512 + NTS + 1],
                    func=Act.Copy,
                )
                # sum(y^2) over the whole row in one DVE instruction
                yflat = y16.rearrange("p t c -> p (t c)")
                nc.vector.scalar_tensor_tensor(
                    out=y2s,
                    in0=yflat,
                    scalar=1.0,
                    in1=yflat,
                    op0=Alu.mult,
                    op1=Alu.mult,
                    accum_out=st[:, 0:1],
                )
                state[k] = {"y16": y16, "st": st}

            def emit_agg(k):
                # tiny PE matmul: group-average [sum(y^2), 3136*mean]
                half = (k % 2) * 16
                gps = ps[:, 7 * 512 + half : 7 * 512 + half + 2]
                nc.tensor.matmul(
                    gps, agg_sb, state[k]["st"], start=True, stop=True,
                    skip_group_check=True,
                )
                state[k]["gps"] = gps

            def emit_chain(k):
                b, oc = divmod(k, OC)
                gps = state[k]["gps"]  # [:,0]=E[y^2]_g  [:,1]=mean_g
                mg = small.tile([128, 1], f32, tag="mg")
                nc.scalar.activation(out=mg, in_=gps[:, 1:2], func=Act.Copy)
                m2 = small.tile([128, 1], f32, tag="m2")
                nc.vector.tensor_mul(m2, gps[:, 1:2], gps[:, 1:2])
                vg = small.tile([128, 1], f32, tag="vg")
                nc.vector.tensor_sub(vg, gps[:, 0:1], m2)
                sd = small.tile([128, 1], f32, tag="sd")
                nc.scalar.activation(out=sd, in_=vg, func=Act.Sqrt, bias=eps_sb)
                rstd = small.tile([128, 1], f32, tag="rstd")
                nc.vector.reciprocal(rstd, sd)
                s = small.tile([128, 1], f32, tag="s")
                nc.vector.tensor_mul(s, rstd, gamma_sb[:, oc : oc + 1])
                # bv = beta63 - mean*s (for the ACT-affine form y*s + bv)
                ms = small.tile([128, 1], f32, tag="ms")
                nc.gpsimd.tensor_mul(ms, mg, s)
                bv = small.tile([128, 1], f32, tag="bv")
                nc.gpsimd.tensor_sub(bv, beta_sb[:, oc : oc + 1], ms)
                state[k].update(mg=mg, s=s, bv=bv)

            def emit_pass3_store(k):
                b, oc = divmod(k, OC)
                osl = slice(oc * 128, (oc + 1) * 128)
                stt = state.pop(k)
                y16, s, bv, mg = stt["y16"], stt["s"], stt["bv"], stt["mg"]
                y8 = y8p.tile([128, HW], i8, tag="y8")
                for eng, banks in P3_GROUPS:
                    lo, hi = banks[0] * NTS, (banks[-1] + 1) * NTS
                    src = y16[:, banks[0] : banks[-1] + 1, :]
                    dst = y8[:, lo:hi].rearrange("p (t c) -> p t c", c=NTS)
                    if eng == "scalar":
                        nc.scalar.activation(
                            out=dst, in_=src, func=Act.Identity, bias=bv,
                            scale=s,
                        )
                    else:
                        e = nc.vector if eng == "vector" else nc.gpsimd
                        e.tensor_scalar(
                            out=dst, in0=src, scalar1=mg, scalar2=s,
                            op0=Alu.subtract, op1=Alu.mult,
                        )
                nc.sync.dma_start(out=out_d.ap()[b, osl, :], in_=y8)

            # ---- main software-pipelined loop --------------------------
            for k in range(NCHUNK):
                b, oc = divmod(k, OC)
                # prefetch next sample's x, one quarter per chunk
                if b + 1 < BPC:
                    if oc == 0:
                        x_tiles.append(xp.tile([128, KC, HW1], f16, tag="x", name=f"x{b + 1}"))
                    load_x_quarter(x_tiles[b + 1], b + 1, oc)
                emit_matmuls(k)
                if STAGE >= 2:
                    emit_pass1(k)
                if k > 0 and STAGE >= 3:
                    emit_agg(k - 1)
                    emit_chain(k - 1)
                    if STAGE >= 4:
                        emit_pass3_store(k - 1)
            if STAGE >= 3:
                emit_agg(NCHUNK - 1)
                emit_chain(NCHUNK - 1)
                if STAGE >= 4:
                    emit_pass3_store(NCHUNK - 1)

    nc.compile()
    return nc


def _get_program():
    global _NC_CACHE
    if _NC_CACHE is None:
        _NC_CACHE = _build_program()
    return _NC_CACHE


def _make_in_maps(x, weight, gamma, beta):
    xr = np.asarray(x, dtype=np.float32).reshape(B, CIN, HW)
    x16 = np.empty((B, CIN, HW + 1), dtype=np.float16)
    x16[:, :, :HW] = xr.astype(np.float16)
    x16[:, :, HW] = x16[:, :, :HW].astype(np.float32).sum(axis=2)
    wt = np.ascontiguousarray(np.asarray(weight, np.float32).T).astype(
        np.float16
    )
    g63 = (np.asarray(gamma, np.float32) * QS).astype(np.float32)
    b63 = (np.asarray(beta, np.float32) * QS).astype(np.float32)
    agg = np.zeros((128, 128), dtype=np.float32)
    for g in range(128 // GSIZE):
        agg[g * GSIZE : (g + 1) * GSIZE, g * GSIZE : (g + 1) * GSIZE] = 1.0 / (
            GSIZE * HW
        )
    return [
        {
            "x": x16[i * BPC : (i + 1) * BPC],
            "wt": wt,
            "gamma": g63,
            "beta": b63,
            "agg": agg,
        }
        for i in range(N_CORES)
    ]


def kernel(x, weight, gamma, beta):
    x = np.asarray(x)
    assert x.shape == (B, CIN, H, W)
    nc = _get_program()
    in_maps = _make_in_maps(x, weight, gamma, beta)
    res = run_bass_kernel_spmd(nc, in_maps, core_ids=list(range(N_CORES)))
    q = np.concatenate([r["out"] for r in res.results], axis=0)
    out = np.clip(q.astype(np.float32) / QS, -2.0, 2.0)
    return out.reshape(B, COUT, H, W)
